# revision 7
# baseline (speedup 1.0000x reference)
"""CondConv (MoE routed conv) Trainium2 Bass kernel.

Strategy (8 NeuronCores, data-parallel over batch, 2 samples/core):
  - All conv data (x, expert slabs, combined weights cw) in bf16.
  - PE does conv + the startup-critical combines via the diagonal trick:
    (s0,ot0,it0) hidden in the DMA wait window, (s1,ot0) mid-stream.
  - DVE: s0 GAP partials, (s0,ot0,it1) combine (gates phase-B),
    (s0,ot1) combine. GpSimd: GAP partials, z1, (s1,ot1) combine.
  - ACT: sigmoids, diag tiles, PSUM->cw copies, BN+SiLU epilogues, and
    output DMAs on the scalar ring (in-order with the epilogue ACT).
  - DMA order: x(s0), ot0/it0 slabs, ot0/it1, x(s1), ot1 slabs; ident
    and small tensors on the gpsimd ring so x starts immediately.
  - Phase-A/B tap-outer in khkw order: conv starts on chunk 0 of cw
    while chunks 1-2 still copy; stationary reuse across blocks.
  - Junk bf16 matmuls on a memset tile bridge PE idle at the start to
    hold the HAM clock-gate at full rate.
"""

import sys

sys.path.insert(0, "/opt/trn_rl_repo")

import numpy as np
import ml_dtypes

import concourse.bass as bass  # noqa: F401
import concourse.mybir as mybir
import concourse.tile as tile
from concourse import bacc
from concourse.bass_utils import run_bass_kernel_spmd

F32 = mybir.dt.float32
BF16 = mybir.dt.bfloat16
AF = mybir.ActivationFunctionType
ALU = mybir.AluOpType

B, CIN, H, W = 16, 256, 56, 56
E, COUT, KS = 8, 256, 3
NCORES = 8
SPC = B // NCORES
IT = CIN // 128
OT = COUT // 128
KHKW = KS * KS
HB = 8  # 7 h-blocks of 8 rows, N = 448
WP = W + 2
PIX = H * W
BN_EPS = 1e-5
SLAB = KHKW * 128  # 1152
CHUNK = 384
NCH = SLAB // CHUNK
NPA = 5  # phase-A open PSUM groups (= psc pool size)
HALF = H * WP // 2  # 1624, GAP partial-reduce split

_PROGRAM_CACHE = {}


def _build_program():
    nc = bacc.Bacc("TRN2", target_bir_lowering=False, debug=False)

    x_d = nc.dram_tensor("x", [SPC, IT, 128, H, WP], BF16, kind="ExternalInput")
    wt_d = nc.dram_tensor("wt", [E, OT, IT, 128, SLAB], BF16, kind="ExternalInput")
    rwt_d = nc.dram_tensor("rwt", [IT, 128, E], F32, kind="ExternalInput")
    rb_d = nc.dram_tensor("rb", [1, E], F32, kind="ExternalInput")
    ident_d = nc.dram_tensor("ident", [128, 128], BF16, kind="ExternalInput")
    bns_d = nc.dram_tensor("bns", [OT, 128, 1], F32, kind="ExternalInput")
    bnb_d = nc.dram_tensor("bnb", [OT, 128, 1], F32, kind="ExternalInput")
    y_d = nc.dram_tensor("y", [SPC, OT, 128, H, W], F32, kind="ExternalOutput")

    with tile.TileContext(nc) as tc:
        with (
            tc.tile_pool(name="xp", bufs=1) as xp,
            tc.tile_pool(name="cwp", bufs=1) as cwp,
            tc.tile_pool(name="wtp", bufs=32) as wtp,
            tc.tile_pool(name="outp", bufs=4) as outp,
            tc.tile_pool(name="smal", bufs=1) as smal,
            tc.tile_pool(name="psc", bufs=NPA, space="PSUM") as psc,
            tc.tile_pool(name="psk", bufs=2, space="PSUM") as psk,
            tc.tile_pool(name="pss", bufs=1, space="PSUM") as pss,
        ):
            # ---- sync ring: x(s0), ot0 slabs, x(s1), ot1 slabs ----
            x_sb = {}

            def load_x(s):
                for it in range(IT):
                    t = xp.tile(
                        [128, H, WP], BF16, tag=f"x_{s}_{it}", name=f"x_{s}_{it}"
                    )
                    nc.sync.dma_start(t[:], x_d[s, it])
                    x_sb[s, it] = t

            slab_tiles = {}

            def load_slabs(ot, its):
                for it in its:
                    for e in range(E):
                        wt_t = wtp.tile(
                            [128, SLAB], BF16, tag="wt", name=f"wt{ot}{it}{e}"
                        )
                        nc.sync.dma_start(wt_t[:], wt_d[e, ot, it])
                        slab_tiles[ot, it, e] = wt_t

            load_x(0)
            load_slabs(0, [0])
            load_slabs(0, [1])
            load_x(1)
            load_slabs(1, range(IT))

            # ---- gpsimd ring: ident + small loads ----
            ident_sb = smal.tile([128, 128], BF16, tag="ident")
            nc.gpsimd.dma_start(ident_sb[:], ident_d[:])
            rwt_sb = []
            for it in range(IT):
                t = smal.tile([128, E], F32, tag=f"rwt{it}", name=f"rwt{it}")
                nc.gpsimd.dma_start(t[:], rwt_d[it])
                rwt_sb.append(t)
            rb_sb = smal.tile([1, E], F32, tag="rb")
            nc.gpsimd.dma_start(rb_sb[:], rb_d[:])
            bns_sb, bnb_sb = [], []
            for ot in range(OT):
                ts_ = smal.tile([128, 1], F32, tag=f"bns{ot}", name=f"bns{ot}")
                nc.gpsimd.dma_start(ts_[:], bns_d[ot])
                bns_sb.append(ts_)
                tb_ = smal.tile([128, 1], F32, tag=f"bnb{ot}", name=f"bnb{ot}")
                nc.gpsimd.dma_start(tb_[:], bnb_d[ot])
                bnb_sb.append(tb_)

            # junk tile for warmup (memset: no DMA dependency)
            jnk_sb = smal.tile([128, CHUNK], BF16, tag="jnk")
            nc.vector.memset(jnk_sb[:], 0.5)
            ones_sb = smal.tile([1, 128], F32, tag="ones")
            nc.vector.memset(ones_sb[:], 1.0)

            def warmup(n):
                # junk bf16 matmuls keep the PE HAM clock-gate at K=8/8
                for _ in range(n):
                    wps = psk.tile([128, CHUNK], F32, tag="kps", name="wps")
                    nc.tensor.matmul(
                        wps[:], jnk_sb[:, 0:128], jnk_sb[:], start=True, stop=True
                    )

            # ---- routing pieces ----
            pooled = {}
            rrow = {}
            r_bcast = {}
            diag = {}

            def routing_reduce_split(s):
                # half-tile partial GAP reduces on DVE + ACT in parallel
                for it in range(IT):
                    flat = x_sb[s, it][:].rearrange("p a b -> p (a b)")
                    for h in range(2):
                        p = smal.tile(
                            [128, 1], F32,
                            tag=f"pool{s}{it}{h}", name=f"pool{s}{it}{h}",
                        )
                        half = flat[:, h * HALF : (h + 1) * HALF]
                        if h == 0:
                            nc.vector.reduce_sum(
                                p[:], half, axis=mybir.AxisListType.X
                            )
                        else:
                            # in-place ACT copy with accum_out
                            nc.scalar.activation(
                                half, half, AF.Copy, accum_out=p[:]
                            )
                        pooled[s, it, h] = p

            def routing_logits_pe(s):
                lg_ps = pss.tile([1, E], F32, tag="rps", name=f"lgps{s}")
                parts = [(it, h) for it in range(IT) for h in range(2)]
                for i, (it, h) in enumerate(parts):
                    nc.tensor.matmul(
                        lg_ps[:], pooled[s, it, h][:], rwt_sb[it][:],
                        start=(i == 0), stop=(i == len(parts) - 1),
                    )
                return lg_ps

            def routing_z(s, lg_ps, eng):
                zr = smal.tile([1, E], F32, tag=f"z{s}", name=f"z{s}")
                eng.scalar_tensor_tensor(
                    zr[:], lg_ps[:], 1.0 / PIX, rb_sb[:], ALU.mult, ALU.add
                )
                rr = smal.tile([1, E], F32, tag=f"r{s}", name=f"r{s}")
                nc.scalar.activation(rr[:], zr[:], AF.Sigmoid)
                rrow[s] = rr

            def routing_bcast_pe(s, eng):
                rb_ps = pss.tile([128, E], F32, tag="rps", name=f"rbps{s}")
                nc.tensor.matmul(rb_ps[:], ones_sb[:], rrow[s][:], start=True, stop=True)
                rbc = smal.tile([128, E], F32, tag=f"rbc{s}", name=f"rbc{s}")
                eng.tensor_copy(rbc[:], rb_ps[:])
                r_bcast[s] = rbc

            def make_diag(s):
                for e in range(E):
                    dt_ = smal.tile(
                        [128, 128], BF16, tag=f"diag{s}{e}", name=f"diag{s}{e}"
                    )
                    nc.scalar.activation(
                        dt_[:], ident_sb[:], AF.Copy,
                        scale=r_bcast[s][:, e : e + 1],
                    )
                    diag[s, e] = dt_

            cw_r = {
                (s, it, ot): cwp.tile(
                    [128, SLAB], BF16,
                    tag=f"cwr_{s}_{it}_{ot}", name=f"cwr_{s}_{it}_{ot}",
                )
                for s in range(SPC)
                for it in range(IT)
                for ot in range(OT)
            }

            def combine_pe(ot, s, its):
                # PE diag trick, chunk-outer/expert-inner; ACT copies PSUM->cw
                for it in its:
                    for c in range(NCH):
                        kps = psk.tile([128, CHUNK], F32, tag="kps", name="kps")
                        for e in range(E):
                            nc.tensor.matmul(
                                kps[:],
                                diag[s, e][:],
                                slab_tiles[ot, it, e][:, c * CHUNK : (c + 1) * CHUNK],
                                start=(e == 0),
                                stop=(e == E - 1),
                            )
                        nc.scalar.activation(
                            cw_r[s, it, ot][:, c * CHUNK : (c + 1) * CHUNK],
                            kps[:],
                            AF.Copy,
                        )

            def combine_eng(eng, ot, s, its):
                # multiply-accumulate chains, expert-major (slab-arrival paced)
                for it in its:
                    dst = cw_r[s, it, ot]
                    for e in range(E):
                        wt_t = slab_tiles[ot, it, e]
                        sc = r_bcast[s][:, e : e + 1]
                        if e == 0:
                            eng.tensor_scalar_mul(dst[:], wt_t[:], sc)
                        else:
                            eng.scalar_tensor_tensor(
                                dst[:], wt_t[:], sc, dst[:], ALU.mult, ALU.add
                            )

            hblocks = [(h0, min(HB, H - h0)) for h0 in range(0, H, HB)]
            # khkw (= chunk-major) tap order
            taps = [(dh, dw) for dh in (-1, 0, 1) for dw in (-1, 0, 1)]

            def block_total(h0, nh):
                return IT * sum(
                    1 for dh, dw in taps if min(h0 + nh, H - dh) > max(h0, -dh)
                )

            def emit_tap(ot, s, it, dh, dw, h0, nh, ps_t, n_mm, total):
                khkw = (dh + 1) * 3 + (dw + 1)
                ho_s = max(h0, -dh)
                ho_e = min(h0 + nh, H - dh)
                if ho_e <= ho_s:
                    return n_mm
                nhh = ho_e - ho_s
                hi_s = ho_s + dh
                off = khkw * 128
                lhsT = cw_r[s, it, ot][:, off : off + 128]
                rhs = x_sb[s, it][:, hi_s : hi_s + nhh, 1 + dw : 1 + dw + W]
                out = ps_t[:, ho_s - h0 : ho_s - h0 + nhh, 0:W]
                nc.tensor.matmul(
                    out, lhsT, rhs,
                    start=(n_mm == 0), stop=(n_mm == total - 1),
                )
                return n_mm + 1

            def conv_taps_outer(ot, s, its, blocks, state):
                # tap-outer (khkw order): stationary reuse + chunk-paced start
                for it in its:
                    for dh, dw in taps:
                        for bi in blocks:
                            h0, nh, ps_t = state[bi][:3]
                            state[bi][3] = emit_tap(
                                ot, s, it, dh, dw, h0, nh, ps_t,
                                state[bi][3], state[bi][4],
                            )

            def conv_epilogue(ot, s, h0, nh, ps_t):
                o_t = outp.tile([128, HB, W], F32, tag="out", name="o_t")
                nc.scalar.activation(
                    o_t[:, :nh, :], ps_t[:, :nh, :], AF.Silu,
                    bias=bnb_sb[ot][:], scale=bns_sb[ot][:],
                )
                nc.scalar.dma_start(y_d[s, ot, :, h0 : h0 + nh, :], o_t[:, :nh, :])

            def conv_block_taps(ot, s, h0, nh, ps_t, its, n_mm, total):
                for it in its:
                    for dh, dw in taps:
                        n_mm = emit_tap(ot, s, it, dh, dw, h0, nh, ps_t, n_mm, total)
                return n_mm

            def conv_full_block(ot, s, h0, nh):
                ps_t = psc.tile([128, HB, W], F32, tag="ps", name="ps")
                total = block_total(h0, nh)
                n_mm = conv_block_taps(ot, s, h0, nh, ps_t, range(IT), 0, total)
                assert n_mm == total
                conv_epilogue(ot, s, h0, nh, ps_t)

            # ================= emission schedule =================
            warmup(34)
            routing_reduce_split(0)
            lg0 = routing_logits_pe(0)
            routing_z(0, lg0, nc.vector)
            routing_bcast_pe(0, nc.vector)
            make_diag(0)
            warmup(6)

            combine_pe(0, 0, [0])
            combine_eng(nc.vector, 0, 0, [1])

            # phase-A: it0 taps, tap-outer, first NPA blocks of (s0, ot0)
            state = {}
            for bi, (h0, nh) in enumerate(hblocks[:NPA]):
                ps_t = psc.tile([128, HB, W], F32, tag="ps", name="ps")
                state[bi] = [h0, nh, ps_t, 0, block_total(h0, nh)]
            conv_taps_outer(0, 0, [0], list(range(NPA)), state)

            routing_reduce_split(1)

            # phase-B: close phase-A blocks with it1 taps
            conv_taps_outer(0, 0, [1], list(range(NPA)), state)
            for bi in range(NPA):
                h0, nh, ps_t, n_mm, total = state[bi]
                assert n_mm == total
                conv_epilogue(0, 0, h0, nh, ps_t)

            lg1 = routing_logits_pe(1)
            routing_z(1, lg1, nc.vector)
            routing_bcast_pe(1, nc.vector)
            make_diag(1)

            # s1/ot0: it0 on PE (copies overlap blocks 5-6), it1 on DVE
            combine_pe(0, 1, [0])
            combine_eng(nc.vector, 0, 1, [1])
            combine_eng(nc.vector, 1, 0, range(IT))
            combine_eng(nc.vector, 1, 1, range(IT))

            # remaining s0/ot0 blocks, then conv s1/ot0, conv ot1
            for h0, nh in hblocks[NPA:]:
                conv_full_block(0, 0, h0, nh)
            for h0, nh in hblocks:
                conv_full_block(0, 1, h0, nh)
            for s in range(SPC):
                for h0, nh in hblocks:
                    conv_full_block(1, s, h0, nh)

    nc.compile()
    return nc


def _get_program():
    if "nc" not in _PROGRAM_CACHE:
        _PROGRAM_CACHE["nc"] = _build_program()
    return _PROGRAM_CACHE["nc"]


def kernel(x, routing_w, routing_b, kernel_weights, bn_gamma, bn_beta, bn_mean, bn_var,
           _trace=False, _trace_kwargs=None):
    x = np.asarray(x, dtype=np.float32)
    routing_w = np.asarray(routing_w, dtype=np.float32)
    routing_b = np.asarray(routing_b, dtype=np.float32)
    kernel_weights = np.asarray(kernel_weights, dtype=np.float32)
    bn_gamma = np.asarray(bn_gamma, dtype=np.float32)
    bn_beta = np.asarray(bn_beta, dtype=np.float32)
    bn_mean = np.asarray(bn_mean, dtype=np.float32)
    bn_var = np.asarray(bn_var, dtype=np.float32)

    bf16 = ml_dtypes.bfloat16
    # wt[e, ot, it, i, khkw*128 + o_in] from kernel_weights[e, o, i, kh, kw]
    kw7 = kernel_weights.reshape(E, OT, 128, IT, 128, KS, KS)
    wt_host = np.ascontiguousarray(kw7.transpose(0, 1, 3, 4, 5, 6, 2)).reshape(
        E, OT, IT, 128, SLAB
    ).astype(bf16)
    rwt_host = np.ascontiguousarray(routing_w.T).reshape(IT, 128, E)
    rb_host = np.ascontiguousarray(routing_b).reshape(1, E)
    ident_host = np.eye(128, dtype=np.float32).astype(bf16)
    inv = bn_gamma / np.sqrt(bn_var + BN_EPS)
    bns_host = np.ascontiguousarray(inv).reshape(OT, 128, 1)
    bnb_host = np.ascontiguousarray(bn_beta - bn_mean * inv).reshape(OT, 128, 1)

    x_pad = np.zeros((B, CIN, H, WP), dtype=np.float32)
    x_pad[:, :, :, 1 : 1 + W] = x
    x_pad = x_pad.astype(bf16)
    in_maps = []
    for g in range(NCORES):
        xg = np.ascontiguousarray(
            x_pad[g * SPC : (g + 1) * SPC].reshape(SPC, IT, 128, H, WP)
        )
        in_maps.append(
            {
                "x": xg,
                "wt": wt_host,
                "rwt": rwt_host,
                "rb": rb_host,
                "ident": ident_host,
                "bns": bns_host,
                "bnb": bnb_host,
            }
        )

    nc = _get_program()
    res = run_bass_kernel_spmd(
        nc, in_maps, core_ids=list(range(NCORES)),
        trace=_trace, **(_trace_kwargs or {}),
    )
    _PROGRAM_CACHE["last_result"] = res

    out = np.empty((B, COUT, H, W), dtype=np.float32)
    for g in range(NCORES):
        yg = res.results[g]["y"]
        out[g * SPC : (g + 1) * SPC] = yg.reshape(SPC, COUT, H, W)
    return out


# revision 9
# speedup vs baseline: 1.0047x; 1.0047x over previous
"""CondConv (MoE routed conv) Trainium2 Bass kernel.

Strategy (8 NeuronCores, data-parallel over batch, 2 samples/core):
  - All conv data (x, expert slabs, combined weights cw) in bf16.
  - PE does conv + the startup-critical combines via the diagonal trick:
    (s0,ot0,it0) hidden in the DMA wait window, (s1,ot0) mid-stream.
  - DVE: s0 GAP partials, (s0,ot0,it1) combine (gates phase-B),
    (s0,ot1) combine. GpSimd: GAP partials, z1, (s1,ot1) combine.
  - ACT: sigmoids, diag tiles, PSUM->cw copies, BN+SiLU epilogues, and
    output DMAs on the scalar ring (in-order with the epilogue ACT).
  - DMA order: x(s0), ot0/it0 slabs, ot0/it1, x(s1), ot1 slabs; ident
    and small tensors on the gpsimd ring so x starts immediately.
  - Phase-A/B tap-outer in khkw order: conv starts on chunk 0 of cw
    while chunks 1-2 still copy; stationary reuse across blocks.
  - Junk bf16 matmuls on a memset tile bridge PE idle at the start to
    hold the HAM clock-gate at full rate.
"""

import sys

sys.path.insert(0, "/opt/trn_rl_repo")

import numpy as np
import ml_dtypes

import concourse.bass as bass  # noqa: F401
import concourse.mybir as mybir
import concourse.tile as tile
from concourse import bacc
from concourse.bass_utils import run_bass_kernel_spmd

F32 = mybir.dt.float32
BF16 = mybir.dt.bfloat16
AF = mybir.ActivationFunctionType
ALU = mybir.AluOpType

B, CIN, H, W = 16, 256, 56, 56
E, COUT, KS = 8, 256, 3
NCORES = 8
SPC = B // NCORES
IT = CIN // 128
OT = COUT // 128
KHKW = KS * KS
HB = 8  # 7 h-blocks of 8 rows, N = 448
WP = W + 2
PIX = H * W
BN_EPS = 1e-5
SLAB = KHKW * 128  # 1152
CHUNK = 384
NCH = SLAB // CHUNK
NPA = 5  # phase-A open PSUM groups (= psc pool size)
HALF = H * WP // 2  # 1624, GAP partial-reduce split

_PROGRAM_CACHE = {}


def _build_program():
    nc = bacc.Bacc("TRN2", target_bir_lowering=False, debug=False)

    x_d = nc.dram_tensor("x", [SPC, IT, 128, H, WP], BF16, kind="ExternalInput")
    wt_d = nc.dram_tensor("wt", [E, OT, IT, 128, SLAB], BF16, kind="ExternalInput")
    rwt_d = nc.dram_tensor("rwt", [IT, 128, E], F32, kind="ExternalInput")
    rb_d = nc.dram_tensor("rb", [1, E], F32, kind="ExternalInput")
    ident_d = nc.dram_tensor("ident", [128, 128], BF16, kind="ExternalInput")
    bns_d = nc.dram_tensor("bns", [OT, 128, 1], F32, kind="ExternalInput")
    bnb_d = nc.dram_tensor("bnb", [OT, 128, 1], F32, kind="ExternalInput")
    y_d = nc.dram_tensor("y", [SPC, OT, 128, H, W], F32, kind="ExternalOutput")

    with tile.TileContext(nc) as tc:
        with (
            tc.tile_pool(name="xp", bufs=1) as xp,
            tc.tile_pool(name="cwp", bufs=1) as cwp,
            tc.tile_pool(name="wtp", bufs=32) as wtp,
            tc.tile_pool(name="outp", bufs=4) as outp,
            tc.tile_pool(name="smal", bufs=1) as smal,
            tc.tile_pool(name="psc", bufs=NPA, space="PSUM") as psc,
            tc.tile_pool(name="psk", bufs=2, space="PSUM") as psk,
            tc.tile_pool(name="pss", bufs=1, space="PSUM") as pss,
        ):
            # ---- sync ring: x(s0), ot0 slabs, x(s1), ot1 slabs ----
            x_sb = {}

            def load_x(s):
                for it in range(IT):
                    t = xp.tile(
                        [128, H, WP], BF16, tag=f"x_{s}_{it}", name=f"x_{s}_{it}"
                    )
                    nc.sync.dma_start(t[:], x_d[s, it])
                    x_sb[s, it] = t

            slab_tiles = {}

            def load_slabs(ot, its):
                for it in its:
                    for e in range(E):
                        wt_t = wtp.tile(
                            [128, SLAB], BF16, tag="wt", name=f"wt{ot}{it}{e}"
                        )
                        nc.sync.dma_start(wt_t[:], wt_d[e, ot, it])
                        slab_tiles[ot, it, e] = wt_t

            load_x(0)
            load_slabs(0, [0])
            load_slabs(0, [1])
            load_x(1)
            load_slabs(1, range(IT))

            # ---- gpsimd ring: ident + small loads ----
            ident_sb = smal.tile([128, 128], BF16, tag="ident")
            nc.gpsimd.dma_start(ident_sb[:], ident_d[:])
            rwt_sb = []
            for it in range(IT):
                t = smal.tile([128, E], F32, tag=f"rwt{it}", name=f"rwt{it}")
                nc.gpsimd.dma_start(t[:], rwt_d[it])
                rwt_sb.append(t)
            rb_sb = smal.tile([1, E], F32, tag="rb")
            nc.gpsimd.dma_start(rb_sb[:], rb_d[:])
            bns_sb, bnb_sb = [], []
            for ot in range(OT):
                ts_ = smal.tile([128, 1], F32, tag=f"bns{ot}", name=f"bns{ot}")
                nc.gpsimd.dma_start(ts_[:], bns_d[ot])
                bns_sb.append(ts_)
                tb_ = smal.tile([128, 1], F32, tag=f"bnb{ot}", name=f"bnb{ot}")
                nc.gpsimd.dma_start(tb_[:], bnb_d[ot])
                bnb_sb.append(tb_)

            # junk tile for warmup (memset: no DMA dependency)
            jnk_sb = smal.tile([128, CHUNK], BF16, tag="jnk")
            nc.vector.memset(jnk_sb[:], 0.5)
            ones_sb = smal.tile([1, 128], F32, tag="ones")
            nc.vector.memset(ones_sb[:], 1.0)

            def warmup(n):
                # junk bf16 matmuls keep the PE HAM clock-gate at K=8/8
                for _ in range(n):
                    wps = psk.tile([128, CHUNK], F32, tag="kps", name="wps")
                    nc.tensor.matmul(
                        wps[:], jnk_sb[:, 0:128], jnk_sb[:], start=True, stop=True
                    )

            # ---- routing pieces ----
            pooled = {}
            rrow = {}
            r_bcast = {}
            diag = {}

            def routing_reduce_split(s):
                # half-tile partial GAP reduces on DVE + ACT in parallel
                for it in range(IT):
                    flat = x_sb[s, it][:].rearrange("p a b -> p (a b)")
                    for h in range(2):
                        p = smal.tile(
                            [128, 1], F32,
                            tag=f"pool{s}{it}{h}", name=f"pool{s}{it}{h}",
                        )
                        half = flat[:, h * HALF : (h + 1) * HALF]
                        if h == 0:
                            nc.vector.reduce_sum(
                                p[:], half, axis=mybir.AxisListType.X
                            )
                        else:
                            # in-place ACT copy with accum_out
                            nc.scalar.activation(
                                half, half, AF.Copy, accum_out=p[:]
                            )
                        pooled[s, it, h] = p

            def routing_logits_pe(s):
                lg_ps = pss.tile([1, E], F32, tag="rps", name=f"lgps{s}")
                parts = [(it, h) for it in range(IT) for h in range(2)]
                for i, (it, h) in enumerate(parts):
                    nc.tensor.matmul(
                        lg_ps[:], pooled[s, it, h][:], rwt_sb[it][:],
                        start=(i == 0), stop=(i == len(parts) - 1),
                    )
                return lg_ps

            def routing_z(s, lg_ps, eng):
                zr = smal.tile([1, E], F32, tag=f"z{s}", name=f"z{s}")
                eng.scalar_tensor_tensor(
                    zr[:], lg_ps[:], 1.0 / PIX, rb_sb[:], ALU.mult, ALU.add
                )
                rr = smal.tile([1, E], F32, tag=f"r{s}", name=f"r{s}")
                nc.scalar.activation(rr[:], zr[:], AF.Sigmoid)
                rrow[s] = rr

            def routing_bcast_pe(s, eng):
                rb_ps = pss.tile([128, E], F32, tag="rps", name=f"rbps{s}")
                nc.tensor.matmul(rb_ps[:], ones_sb[:], rrow[s][:], start=True, stop=True)
                rbc = smal.tile([128, E], F32, tag=f"rbc{s}", name=f"rbc{s}")
                eng.tensor_copy(rbc[:], rb_ps[:])
                r_bcast[s] = rbc

            def make_diag(s):
                for e in range(E):
                    dt_ = smal.tile(
                        [128, 128], BF16, tag=f"diag{s}{e}", name=f"diag{s}{e}"
                    )
                    nc.scalar.activation(
                        dt_[:], ident_sb[:], AF.Copy,
                        scale=r_bcast[s][:, e : e + 1],
                    )
                    diag[s, e] = dt_

            cw_r = {
                (s, it, ot): cwp.tile(
                    [128, SLAB], BF16,
                    tag=f"cwr_{s}_{it}_{ot}", name=f"cwr_{s}_{it}_{ot}",
                )
                for s in range(SPC)
                for it in range(IT)
                for ot in range(OT)
            }

            def combine_pe(ot, s, its):
                # PE diag trick, chunk-outer/expert-inner; ACT copies PSUM->cw
                for it in its:
                    for c in range(NCH):
                        kps = psk.tile([128, CHUNK], F32, tag="kps", name="kps")
                        for e in range(E):
                            nc.tensor.matmul(
                                kps[:],
                                diag[s, e][:],
                                slab_tiles[ot, it, e][:, c * CHUNK : (c + 1) * CHUNK],
                                start=(e == 0),
                                stop=(e == E - 1),
                            )
                        nc.scalar.activation(
                            cw_r[s, it, ot][:, c * CHUNK : (c + 1) * CHUNK],
                            kps[:],
                            AF.Copy,
                        )

            def combine_eng(eng, ot, s, its):
                # multiply-accumulate chains, expert-major (slab-arrival paced)
                for it in its:
                    dst = cw_r[s, it, ot]
                    for e in range(E):
                        wt_t = slab_tiles[ot, it, e]
                        sc = r_bcast[s][:, e : e + 1]
                        if e == 0:
                            eng.tensor_scalar_mul(dst[:], wt_t[:], sc)
                        else:
                            eng.scalar_tensor_tensor(
                                dst[:], wt_t[:], sc, dst[:], ALU.mult, ALU.add
                            )

            hblocks = [(h0, min(HB, H - h0)) for h0 in range(0, H, HB)]
            # khkw (= chunk-major) tap order
            taps = [(dh, dw) for dh in (-1, 0, 1) for dw in (-1, 0, 1)]

            def block_total(h0, nh):
                return IT * sum(
                    1 for dh, dw in taps if min(h0 + nh, H - dh) > max(h0, -dh)
                )

            def emit_tap(ot, s, it, dh, dw, h0, nh, ps_t, n_mm, total):
                khkw = (dh + 1) * 3 + (dw + 1)
                ho_s = max(h0, -dh)
                ho_e = min(h0 + nh, H - dh)
                if ho_e <= ho_s:
                    return n_mm
                nhh = ho_e - ho_s
                hi_s = ho_s + dh
                off = khkw * 128
                lhsT = cw_r[s, it, ot][:, off : off + 128]
                rhs = x_sb[s, it][:, hi_s : hi_s + nhh, 1 + dw : 1 + dw + W]
                out = ps_t[:, ho_s - h0 : ho_s - h0 + nhh, 0:W]
                nc.tensor.matmul(
                    out, lhsT, rhs,
                    start=(n_mm == 0), stop=(n_mm == total - 1),
                )
                return n_mm + 1

            def conv_taps_outer(ot, s, its, blocks, state):
                # tap-outer (khkw order): stationary reuse + chunk-paced start
                for it in its:
                    for dh, dw in taps:
                        for bi in blocks:
                            h0, nh, ps_t = state[bi][:3]
                            state[bi][3] = emit_tap(
                                ot, s, it, dh, dw, h0, nh, ps_t,
                                state[bi][3], state[bi][4],
                            )

            def conv_epilogue(ot, s, h0, nh, ps_t):
                o_t = outp.tile([128, HB, W], F32, tag="out", name="o_t")
                nc.scalar.activation(
                    o_t[:, :nh, :], ps_t[:, :nh, :], AF.Silu,
                    bias=bnb_sb[ot][:], scale=bns_sb[ot][:],
                )
                nc.scalar.dma_start(y_d[s, ot, :, h0 : h0 + nh, :], o_t[:, :nh, :])

            def conv_block_taps(ot, s, h0, nh, ps_t, its, n_mm, total):
                for it in its:
                    for dh, dw in taps:
                        n_mm = emit_tap(ot, s, it, dh, dw, h0, nh, ps_t, n_mm, total)
                return n_mm

            def conv_full_block(ot, s, h0, nh):
                ps_t = psc.tile([128, HB, W], F32, tag="ps", name="ps")
                total = block_total(h0, nh)
                n_mm = conv_block_taps(ot, s, h0, nh, ps_t, range(IT), 0, total)
                assert n_mm == total
                conv_epilogue(ot, s, h0, nh, ps_t)

            # ================= emission schedule =================
            warmup(34)
            routing_reduce_split(0)
            lg0 = routing_logits_pe(0)
            routing_z(0, lg0, nc.vector)
            routing_bcast_pe(0, nc.vector)
            make_diag(0)
            warmup(24)

            combine_pe(0, 0, [0])
            combine_eng(nc.vector, 0, 0, [1])

            # phase-A: it0 taps, tap-outer, first NPA blocks of (s0, ot0)
            state = {}
            for bi, (h0, nh) in enumerate(hblocks[:NPA]):
                ps_t = psc.tile([128, HB, W], F32, tag="ps", name="ps")
                state[bi] = [h0, nh, ps_t, 0, block_total(h0, nh)]
            conv_taps_outer(0, 0, [0], list(range(NPA)), state)

            routing_reduce_split(1)

            # phase-B: close phase-A blocks with it1 taps
            conv_taps_outer(0, 0, [1], list(range(NPA)), state)
            for bi in range(NPA):
                h0, nh, ps_t, n_mm, total = state[bi]
                assert n_mm == total
                conv_epilogue(0, 0, h0, nh, ps_t)

            lg1 = routing_logits_pe(1)
            routing_z(1, lg1, nc.vector)
            routing_bcast_pe(1, nc.vector)
            make_diag(1)

            # s1/ot0: it0 on PE (copies overlap blocks 5-6), it1 on DVE
            combine_pe(0, 1, [0])
            combine_eng(nc.vector, 0, 1, [1])
            combine_eng(nc.vector, 1, 0, [1])
            combine_eng(nc.vector, 1, 1, range(IT))

            # remaining s0/ot0 blocks, then conv s1/ot0, conv ot1
            for h0, nh in hblocks[NPA:]:
                conv_full_block(0, 0, h0, nh)
            for h0, nh in hblocks:
                conv_full_block(0, 1, h0, nh)
            combine_pe(1, 0, [0])  # (s0, ot1, it0) on PE between conv phases
            for h0, nh in hblocks:
                conv_full_block(1, 0, h0, nh)
            for h0, nh in hblocks:
                conv_full_block(1, 1, h0, nh)

    nc.compile()
    return nc


def _get_program():
    if "nc" not in _PROGRAM_CACHE:
        _PROGRAM_CACHE["nc"] = _build_program()
    return _PROGRAM_CACHE["nc"]


def kernel(x, routing_w, routing_b, kernel_weights, bn_gamma, bn_beta, bn_mean, bn_var,
           _trace=False, _trace_kwargs=None):
    x = np.asarray(x, dtype=np.float32)
    routing_w = np.asarray(routing_w, dtype=np.float32)
    routing_b = np.asarray(routing_b, dtype=np.float32)
    kernel_weights = np.asarray(kernel_weights, dtype=np.float32)
    bn_gamma = np.asarray(bn_gamma, dtype=np.float32)
    bn_beta = np.asarray(bn_beta, dtype=np.float32)
    bn_mean = np.asarray(bn_mean, dtype=np.float32)
    bn_var = np.asarray(bn_var, dtype=np.float32)

    bf16 = ml_dtypes.bfloat16
    # wt[e, ot, it, i, khkw*128 + o_in] from kernel_weights[e, o, i, kh, kw]
    kw7 = kernel_weights.reshape(E, OT, 128, IT, 128, KS, KS)
    wt_host = np.ascontiguousarray(kw7.transpose(0, 1, 3, 4, 5, 6, 2)).reshape(
        E, OT, IT, 128, SLAB
    ).astype(bf16)
    rwt_host = np.ascontiguousarray(routing_w.T).reshape(IT, 128, E)
    rb_host = np.ascontiguousarray(routing_b).reshape(1, E)
    ident_host = np.eye(128, dtype=np.float32).astype(bf16)
    inv = bn_gamma / np.sqrt(bn_var + BN_EPS)
    bns_host = np.ascontiguousarray(inv).reshape(OT, 128, 1)
    bnb_host = np.ascontiguousarray(bn_beta - bn_mean * inv).reshape(OT, 128, 1)

    x_pad = np.zeros((B, CIN, H, WP), dtype=np.float32)
    x_pad[:, :, :, 1 : 1 + W] = x
    x_pad = x_pad.astype(bf16)
    in_maps = []
    for g in range(NCORES):
        xg = np.ascontiguousarray(
            x_pad[g * SPC : (g + 1) * SPC].reshape(SPC, IT, 128, H, WP)
        )
        in_maps.append(
            {
                "x": xg,
                "wt": wt_host,
                "rwt": rwt_host,
                "rb": rb_host,
                "ident": ident_host,
                "bns": bns_host,
                "bnb": bnb_host,
            }
        )

    nc = _get_program()
    res = run_bass_kernel_spmd(
        nc, in_maps, core_ids=list(range(NCORES)),
        trace=_trace, **(_trace_kwargs or {}),
    )
    _PROGRAM_CACHE["last_result"] = res

    out = np.empty((B, COUT, H, W), dtype=np.float32)
    for g in range(NCORES):
        yg = res.results[g]["y"]
        out[g * SPC : (g + 1) * SPC] = yg.reshape(SPC, COUT, H, W)
    return out
